# revision 1
# baseline (speedup 1.0000x reference)
"""Trainium2 Bass kernel for an 8-expert top-2 MoE layer (+ shared expert).

Two-phase sparse expert-parallel strategy across 8 NeuronCores:

Phase 1 (one NEFF, SPMD on all cores):
- Replicated fp32 router: logits = x @ router_w + b, softmax, exact top-2
  selection computed from fp32 logits (max / masked-second-max via is_ge),
  so selection matches the fp32 reference. Emits the combine-weight matrix
  ct [T, E] back to the host.
- Shared expert, sharded over its hidden dim (core e owns hidden units
  [e*512, (e+1)*512)): partial FFN for ALL tokens in token-major layout,
  summed across cores with per-token-slice ReduceScatters.

Host dispatch (the "all-to-all by top-k expert id" step):
- Builds per-expert gather lists from the device-computed ct, pads to
  capacity C, pre-gathers/transposes/casts x per core, and derives
  per-chunk scatter indices. Verifies the static group->chunk schedule
  against the actual routing and falls back to the dense single-phase
  kernel (_build_nc, also selectable via MOE_DENSE=1) if it ever fails.

Phase 2 (one NEFF, SPMD):
- Core e runs expert e's FFN (bf16 matmuls, fp32 PSUM, erf-Gelu) over only
  its ~C routed tokens; outputs are weighted by the combine weights,
  written token-major via bf16 indirect-DMA scatter into 3 row-chunk
  buffers, and each chunk is ReduceScattered as soon as its writers are
  done so the collectives overlap the remaining compute.
- Host reassembles the chunk shards, adds the shared-expert shards and the
  (unweighted) sb2 bias, and reshapes to the reference output.

Everything computes in the transposed-activation layout (tokens on the free
axis) so no on-device transposes are needed; the host pre-permutes x and all
weights into SBUF-ready [ki, ko, free] blocks.
"""

import sys

if "/opt/trn_rl_repo" not in sys.path:
    sys.path.insert(0, "/opt/trn_rl_repo")

import numpy as np
import ml_dtypes

DIM = 1024
E = 8
H = 4096
T = 4096  # B*S = 2*2048 tokens
NCORES = 8
P = 128
DKO = DIM // P     # 8 k-subtiles over dim
HKO = H // P       # 32 k-subtiles over hidden
SH = H // NCORES   # 512 shared-expert hidden slice
SHKO = SH // P     # 4
SLICE = 512        # token slice width
NSLICES = T // SLICE

BF16 = ml_dtypes.bfloat16

_nc_cache = {}


def _build_nc(sim=False):
    import concourse.mybir as mybir
    import concourse.tile as tile
    from concourse import bacc
    from concourse.masks import make_identity

    f32 = mybir.dt.float32
    bf16 = mybir.dt.bfloat16
    AF = mybir.ActivationFunctionType
    OP = mybir.AluOpType
    AX = mybir.AxisListType

    ndev = 1 if sim else NCORES
    nc = bacc.Bacc("TRN2", target_bir_lowering=False, debug=False, num_devices=ndev)

    xtbf = nc.dram_tensor("xtbf", [P, DKO, T], bf16, kind="ExternalInput")
    xt32 = nc.dram_tensor("xt32", [P, DKO, T], f32, kind="ExternalInput")
    rwp = nc.dram_tensor("rwp", [P, DKO, E], f32, kind="ExternalInput")
    rb = nc.dram_tensor("rb", [P, E], f32, kind="ExternalInput")
    w1p = nc.dram_tensor("w1p", [P, DKO, H], bf16, kind="ExternalInput")
    b1c = nc.dram_tensor("b1c", [P, HKO], f32, kind="ExternalInput")
    w2b = nc.dram_tensor("w2b", [DKO, P, HKO, P], bf16, kind="ExternalInput")
    b2c = nc.dram_tensor("b2c", [P, DKO], f32, kind="ExternalInput")
    sw1p = nc.dram_tensor("sw1p", [P, DKO, SH], bf16, kind="ExternalInput")
    sb1c = nc.dram_tensor("sb1c", [P, SHKO], f32, kind="ExternalInput")
    sw2p = nc.dram_tensor("sw2p", [P, SHKO, DIM], bf16, kind="ExternalInput")
    sb2c = nc.dram_tensor("sb2c", [P, DKO], f32, kind="ExternalInput")
    oh = nc.dram_tensor("oh", [E, P], f32, kind="ExternalInput")
    out = nc.dram_tensor("out", [P, T], f32, kind="ExternalOutput")

    with tile.TileContext(nc) as tc:
        with (
            tc.tile_pool(name="const", bufs=1) as const,
            tc.tile_pool(name="wpool", bufs=1) as wpool,
        ):
            ident = const.tile([P, P], f32)
            make_identity(nc, ident)
            rwp_sb = const.tile([P, DKO, E], f32)
            nc.sync.dma_start(rwp_sb, rwp[:, :, :])
            rb_sb = const.tile([P, E], f32)
            nc.sync.dma_start(rb_sb, rb[:, :])
            b1c_sb = const.tile([P, HKO], f32)
            nc.sync.dma_start(b1c_sb, b1c[:, :])
            b2c_sb = const.tile([P, DKO], f32)
            nc.sync.dma_start(b2c_sb, b2c[:, :])
            sb1c_sb = const.tile([P, SHKO], f32)
            nc.sync.dma_start(sb1c_sb, sb1c[:, :])
            sb2c_sb = const.tile([P, DKO], f32)
            nc.sync.dma_start(sb2c_sb, sb2c[:, :])
            oh_sb = const.tile([E, P], f32)
            nc.sync.dma_start(oh_sb, oh[:, :])

            w1_sb = wpool.tile([P, DKO, H], bf16)
            nc.sync.dma_start(w1_sb, w1p[:, :, :])
            sw1_sb = wpool.tile([P, DKO, SH], bf16)
            nc.sync.dma_start(sw1_sb, sw1p[:, :, :])
            sw2_sb = wpool.tile([P, SHKO, DIM], bf16)
            nc.sync.dma_start(sw2_sb, sw2p[:, :, :])
            ct_sb = wpool.tile([E, T], f32)  # transposed combine weights c_e(t)

            # ---- Router: fp32 logits, softmax, top-2 select, transpose ----
            with (
                tc.tile_pool(name="rx", bufs=2) as rxp,
                tc.tile_pool(name="rt", bufs=2) as rt,
                tc.tile_pool(name="rps", bufs=2, space="PSUM") as rps,
                tc.tile_pool(name="tps", bufs=2, space="PSUM") as tps,
            ):
                for tt in range(T // P):
                    rx = rxp.tile([P, DKO, P], f32, tag="rx")
                    nc.sync.dma_start(rx, xt32[:, :, tt * P : (tt + 1) * P])
                    pl = rps.tile([P, E], f32, tag="pl")
                    for ko in range(DKO):
                        nc.tensor.matmul(
                            pl,
                            rx[:, ko, :],
                            rwp_sb[:, ko, :],
                            start=(ko == 0),
                            stop=(ko == DKO - 1),
                        )
                    lg = rt.tile([P, E], f32, tag="lg")
                    nc.vector.tensor_add(lg, pl, rb_sb)
                    mx = rt.tile([P, 1], f32, tag="mx")
                    nc.vector.reduce_max(mx, lg, axis=AX.X)
                    nmx = rt.tile([P, 1], f32, tag="nmx")
                    nc.vector.tensor_scalar_mul(nmx, mx, -1.0)
                    ex = rt.tile([P, E], f32, tag="ex")
                    nc.scalar.activation(ex, lg, AF.Exp, bias=nmx)
                    sm = rt.tile([P, 1], f32, tag="sm")
                    nc.vector.reduce_sum(sm, ex, axis=AX.X)
                    rc = rt.tile([P, 1], f32, tag="rc")
                    nc.vector.reciprocal(rc, sm)
                    ge1 = rt.tile([P, E], f32, tag="ge1")
                    nc.vector.tensor_tensor(ge1, lg, mx.to_broadcast((P, E)), OP.is_ge)
                    big = rt.tile([P, E], f32, tag="big")
                    nc.vector.tensor_scalar_mul(big, ge1, 1e30)
                    lm = rt.tile([P, E], f32, tag="lm")
                    nc.vector.tensor_sub(lm, lg, big)
                    m2 = rt.tile([P, 1], f32, tag="m2")
                    nc.vector.reduce_max(m2, lm, axis=AX.X)
                    msk = rt.tile([P, E], f32, tag="msk")
                    nc.vector.tensor_tensor(msk, lg, m2.to_broadcast((P, E)), OP.is_ge)
                    pw = rt.tile([P, E], f32, tag="pw")
                    nc.vector.tensor_mul(pw, ex, msk)
                    nc.vector.tensor_tensor(pw, pw, rc.to_broadcast((P, E)), OP.mult)
                    cps = tps.tile([E, P], f32, tag="cps")
                    nc.tensor.transpose(cps, pw, ident)
                    nc.vector.tensor_copy(ct_sb[:, tt * P : (tt + 1) * P], cps)

            # ---- Main FFN loop over token slices ----
            with (
                tc.tile_pool(name="xp", bufs=2) as xp,
                tc.tile_pool(name="w2p", bufs=2) as w2p,
                tc.tile_pool(name="hp", bufs=1) as hp,
                tc.tile_pool(name="cep", bufs=1) as cep,
                tc.tile_pool(name="op", bufs=1) as op_,
                tc.tile_pool(name="tp", bufs=1) as tp_,
                tc.tile_pool(name="dram", bufs=1, space="DRAM") as dram,
                tc.tile_pool(name="p1", bufs=2, space="PSUM") as p1p,
                tc.tile_pool(name="p2", bufs=2, space="PSUM") as p2p,
                tc.tile_pool(name="p2s", bufs=1, space="PSUM") as p2sp,
                tc.tile_pool(name="pc", bufs=1, space="PSUM") as pcp,
            ):
                for s in range(NSLICES):
                    t0 = s * SLICE
                    xt = xp.tile([P, DKO, SLICE], bf16, tag="xt")
                    nc.sync.dma_start(xt, xtbf[:, :, t0 : t0 + SLICE])
                    # replicate c_e(t) across all 128 partitions via matmul
                    cei = pcp.tile([P, SLICE], f32, tag="cei")
                    nc.tensor.matmul(
                        cei, oh_sb, ct_sb[:, t0 : t0 + SLICE], start=True, stop=True
                    )
                    ce = cep.tile([P, SLICE], f32, tag="ce")
                    nc.vector.tensor_copy(ce, cei)

                    h = hp.tile([P, HKO + SHKO, SLICE], bf16, tag="h")
                    for hm in range(HKO):
                        ps = p1p.tile([P, SLICE], f32, tag="ps1")
                        for ko in range(DKO):
                            nc.tensor.matmul(
                                ps,
                                w1_sb[:, ko, hm * P : (hm + 1) * P],
                                xt[:, ko, :],
                                start=(ko == 0),
                                stop=(ko == DKO - 1),
                            )
                        nc.scalar.activation(
                            h[:, hm, :], ps, AF.Gelu, bias=b1c_sb[:, hm : hm + 1]
                        )
                    for sm_ in range(SHKO):
                        ps = p1p.tile([P, SLICE], f32, tag="ps1")
                        for ko in range(DKO):
                            nc.tensor.matmul(
                                ps,
                                sw1_sb[:, ko, sm_ * P : (sm_ + 1) * P],
                                xt[:, ko, :],
                                start=(ko == 0),
                                stop=(ko == DKO - 1),
                            )
                        nc.scalar.activation(
                            h[:, HKO + sm_, :], ps, AF.Gelu, bias=sb1c_sb[:, sm_ : sm_ + 1]
                        )

                    ob = op_.tile([P, DKO, SLICE], f32, tag="ob")
                    for dm in range(DKO):
                        w2t = w2p.tile([P, HKO, P], bf16, tag="w2t")
                        nc.sync.dma_start(w2t, w2b[dm, :, :, :])
                        ps2 = p2p.tile([P, SLICE], f32, tag="ps2")
                        for hk in range(HKO):
                            nc.tensor.matmul(
                                ps2,
                                w2t[:, hk, :],
                                h[:, hk, :],
                                start=(hk == 0),
                                stop=(hk == HKO - 1),
                            )
                        ps2s = p2sp.tile([P, SLICE], f32, tag="ps2s")
                        for sk in range(SHKO):
                            nc.tensor.matmul(
                                ps2s,
                                sw2_sb[:, sk, dm * P : (dm + 1) * P],
                                h[:, HKO + sk, :],
                                start=(sk == 0),
                                stop=(sk == SHKO - 1),
                            )
                        t1 = tp_.tile([P, SLICE], f32, tag="t1")
                        nc.scalar.activation(
                            t1, ps2, AF.Identity, bias=b2c_sb[:, dm : dm + 1]
                        )
                        nc.vector.tensor_mul(t1, t1, ce)
                        t2 = tp_.tile([P, SLICE], f32, tag="t2")
                        nc.scalar.activation(
                            t2, ps2s, AF.Identity, bias=sb2c_sb[:, dm : dm + 1]
                        )
                        nc.vector.tensor_add(ob[:, dm, :], t1, t2)

                    obd = dram.tile([DIM, SLICE], f32, tag=f"obd{s}", name=f"obd{s}")
                    nc.sync.dma_start(
                        obd.rearrange("(dm ki) t -> ki dm t", ki=P), ob
                    )
                    rso = dram.tile([P, SLICE], f32, tag=f"rso{s}", name=f"rso{s}")
                    if sim:
                        nc.sync.dma_start(rso[:, :], obd[0:P, :])
                    else:
                        nc.gpsimd.collective_compute(
                            "ReduceScatter",
                            OP.add,
                            replica_groups=[list(range(NCORES))],
                            ins=[obd.opt()],
                            outs=[rso.opt()],
                        )
                    nc.sync.dma_start(out[:, t0 : t0 + SLICE], rso[:, :])

    nc.finalize()
    return nc


def _get_nc():
    if "nc" not in _nc_cache:
        _nc_cache["nc"] = _build_nc()
    return _nc_cache["nc"]


def _prep_in_maps(x, router_w, router_b, w1, b1, w2, b2, sw1, sb1, sw2, sb2):
    xt = np.ascontiguousarray(x.reshape(T, DIM).astype(np.float32).T)  # [DIM, T]
    xt32p = np.ascontiguousarray(xt.reshape(DKO, P, T).transpose(1, 0, 2))
    xtbfp = xt32p.astype(BF16)
    rwp = np.ascontiguousarray(
        router_w.astype(np.float32).reshape(DKO, P, E).transpose(1, 0, 2)
    )
    rb = np.tile(router_b.astype(np.float32)[None, :], (P, 1))
    rb = np.ascontiguousarray(rb)
    in_maps = []
    for e in range(NCORES):
        w1p = np.ascontiguousarray(
            w1[e].reshape(DKO, P, H).transpose(1, 0, 2)
        ).astype(BF16)
        b1ce = np.ascontiguousarray(b1[e].astype(np.float32).reshape(HKO, P).T)
        w2bb = np.ascontiguousarray(
            w2[e].reshape(HKO, P, DKO, P).transpose(2, 1, 0, 3)
        ).astype(BF16)
        b2ce = np.ascontiguousarray(b2[e].astype(np.float32).reshape(DKO, P).T)
        s0 = e * SH
        sw1pe = np.ascontiguousarray(
            sw1[:, s0 : s0 + SH].reshape(DKO, P, SH).transpose(1, 0, 2)
        ).astype(BF16)
        sb1ce = np.ascontiguousarray(
            sb1[s0 : s0 + SH].astype(np.float32).reshape(SHKO, P).T
        )
        sw2pe = np.ascontiguousarray(
            sw2[s0 : s0 + SH, :].reshape(SHKO, P, DIM).transpose(1, 0, 2)
        ).astype(BF16)
        sb2v = sb2 if e == 0 else np.zeros_like(sb2)
        sb2ce = np.ascontiguousarray(sb2v.astype(np.float32).reshape(DKO, P).T)
        ohm = np.zeros((E, P), np.float32)
        ohm[e, :] = 1.0
        in_maps.append(
            dict(
                xtbf=xtbfp,
                xt32=xt32p,
                rwp=rwp,
                rb=rb,
                w1p=w1p,
                b1c=b1ce,
                w2b=w2bb,
                b2c=b2ce,
                sw1p=sw1pe,
                sb1c=sb1ce,
                sw2p=sw2pe,
                sb2c=sb2ce,
                oh=ohm,
            )
        )
    return in_maps


C = 1152                      # per-expert token capacity (actual max 1091)
CT = C // P                   # 9 token tiles of 128
SLICES2 = [512, 512, 128]     # phase-2 token slices (sum = C)
BUFROWS = T + 8               # scatter buffer rows (8 dump rows)
# Chunked return ReduceScatter: token rows are split into 3 chunks so the
# RS for a chunk can run while later slices still compute. Sorted gather
# order means slot group g only ever touches the statically assigned chunks
# below (host verifies; falls back to the unchunked dense path otherwise).
CHUNK_BASE = [0, 1280, 2560]
CHUNK_TOK = [1280, 1280, 1536]          # token rows per chunk
CHUNK_ROWS = [1288, 1288, 1544]         # + 8 dump rows each (div by 8)
GROUP_CHUNKS = [[0], [0], [0, 1], [0, 1], [1, 2], [1, 2], [1, 2], [2], [2]]
GROUP_SLICE = [0, 0, 0, 0, 1, 1, 1, 1, 2]


def _build_phase1(sim=False):
    import concourse.mybir as mybir
    import concourse.tile as tile
    from concourse import bacc
    f32 = mybir.dt.float32
    bf16 = mybir.dt.bfloat16
    AF = mybir.ActivationFunctionType
    OP = mybir.AluOpType
    AX = mybir.AxisListType
    nc = bacc.Bacc("TRN2", target_bir_lowering=False, debug=False,
                   num_devices=1 if sim else NCORES)

    xt32 = nc.dram_tensor("xt32", [P, DKO, T], f32, kind="ExternalInput")
    rwp = nc.dram_tensor("rwp", [P, DKO, E], f32, kind="ExternalInput")
    rb = nc.dram_tensor("rb", [P, E], f32, kind="ExternalInput")
    sw1p = nc.dram_tensor("sw1p", [P, DKO, SH], bf16, kind="ExternalInput")
    sb1c = nc.dram_tensor("sb1c", [P, SHKO], f32, kind="ExternalInput")
    sw2p = nc.dram_tensor("sw2p", [P, SHKO, DIM], bf16, kind="ExternalInput")
    ct_out = nc.dram_tensor("ct", [T, E], f32, kind="ExternalOutput")
    shout = nc.dram_tensor("shout", [NSLICES, T // NSLICES // NCORES, DIM], bf16,
                           kind="ExternalOutput")

    with tile.TileContext(nc) as tc:
        with (
            tc.tile_pool(name="const", bufs=1) as const,
            tc.tile_pool(name="wpool", bufs=1) as wpool,
            tc.tile_pool(name="xp", bufs=3) as xp,
            tc.tile_pool(name="xbp", bufs=3) as xbp,
            tc.tile_pool(name="rt", bufs=3) as rt,
            tc.tile_pool(name="hp", bufs=3) as hp,
            tc.tile_pool(name="osb", bufs=4) as osb,
            tc.tile_pool(name="dram", bufs=1, space="DRAM") as dram,
            tc.tile_pool(name="rps", bufs=2, space="PSUM") as rps,
            tc.tile_pool(name="p1", bufs=2, space="PSUM") as p1p,
            tc.tile_pool(name="p2", bufs=2, space="PSUM") as p2p,
        ):
            rwp_sb = const.tile([P, DKO, E], f32)
            nc.sync.dma_start(rwp_sb, rwp[:, :, :])
            rb_sb = const.tile([P, E], f32)
            nc.sync.dma_start(rb_sb, rb[:, :])
            sb1c_sb = const.tile([P, SHKO], f32)
            nc.sync.dma_start(sb1c_sb, sb1c[:, :])
            sw1_sb = wpool.tile([P, DKO, SH], bf16)
            nc.sync.dma_start(sw1_sb, sw1p[:, :, :])
            sw2_sb = wpool.tile([P, SHKO, DIM], bf16)
            nc.sync.dma_start(sw2_sb, sw2p[:, :, :])

            CH = T // NSLICES // NCORES  # 64 rows per core per slice
            for s in range(NSLICES):
                t0 = s * SLICE
                xt = xp.tile([P, DKO, SLICE], f32, tag="xt")
                for ko2 in range(DKO):
                    nc.sync.dma_start(xt[:, ko2, :],
                                      xt32[:, ko2, t0 : t0 + SLICE])
                xtb = xbp.tile([P, DKO, SLICE], bf16, tag="xtb")
                nc.vector.tensor_copy(xtb, xt)

                # ---- router for this slice (batched over 4 token tiles) ----
                NT4 = SLICE // P
                lg4 = rt.tile([P, NT4, E], f32, tag="lg4")
                for t4 in range(NT4):
                    pl = rps.tile([P, E], f32, tag="pl")
                    for ko in range(DKO):
                        nc.tensor.matmul(pl, xt[:, ko, t4 * P : (t4 + 1) * P],
                                         rwp_sb[:, ko, :],
                                         start=(ko == 0), stop=(ko == DKO - 1))
                    nc.vector.tensor_add(lg4[:, t4, :], pl, rb_sb)
                mx4 = rt.tile([P, NT4, 1], f32, tag="mx4")
                nc.vector.reduce_max(mx4, lg4, axis=AX.X)
                lgs = rt.tile([P, NT4, E], f32, tag="lgs")
                nc.vector.tensor_sub(lgs, lg4, mx4.to_broadcast((P, NT4, E)))
                ex4 = rt.tile([P, NT4, E], f32, tag="ex4")
                nc.scalar.activation(ex4, lgs, AF.Exp)
                sm4 = rt.tile([P, NT4, 1], f32, tag="sm4")
                nc.vector.reduce_sum(sm4, ex4, axis=AX.X)
                rc4 = rt.tile([P, NT4, 1], f32, tag="rc4")
                nc.vector.reciprocal(rc4, sm4)
                ge1 = rt.tile([P, NT4, E], f32, tag="ge1")
                nc.vector.tensor_scalar(ge1, lgs, 0.0, 1e30,
                                        OP.is_ge, OP.mult)
                lm4 = rt.tile([P, NT4, E], f32, tag="lm4")
                nc.vector.tensor_sub(lm4, lgs, ge1)
                m24 = rt.tile([P, NT4, 1], f32, tag="m24")
                nc.vector.reduce_max(m24, lm4, axis=AX.X)
                msk4 = rt.tile([P, NT4, E], f32, tag="msk4")
                nc.vector.tensor_tensor(msk4, lgs, m24.to_broadcast((P, NT4, E)),
                                        OP.is_ge)
                pw4 = rt.tile([P, NT4, E], f32, tag="pw4")
                nc.vector.tensor_mul(pw4, ex4, msk4)
                nc.vector.tensor_tensor(pw4, pw4, rc4.to_broadcast((P, NT4, E)),
                                        OP.mult)
                nc.sync.dma_start(
                    ct_out[s * SLICE : (s + 1) * SLICE, :].rearrange(
                        "(t4 p) e -> p t4 e", p=P),
                    pw4)

                # ---- shared expert (hidden slice) for this token slice ----
                hs = hp.tile([P, SHKO, SLICE], bf16, tag="hs")
                for sm_ in range(SHKO):
                    ps = p1p.tile([P, SLICE], f32, tag="ps1")
                    for ko in range(DKO):
                        nc.tensor.matmul(ps, sw1_sb[:, ko, sm_ * P : (sm_ + 1) * P],
                                         xtb[:, ko, :],
                                         start=(ko == 0), stop=(ko == DKO - 1))
                    nc.scalar.activation(hs[:, sm_, :], ps, AF.Gelu,
                                         bias=sb1c_sb[:, sm_ : sm_ + 1])
                shb = dram.tile([SLICE, DIM], bf16, tag=f"shb{s}", name=f"shb{s}")
                for tt in range(SLICE // P):
                    o_sb = osb.tile([P, DIM], bf16, tag="o_sb")
                    ps2a = p2p.tile([P, 512], f32, tag="ps2_0")
                    ps2b = p2p.tile([P, 512], f32, tag="ps2_1")
                    for sk in range(SHKO):
                        nc.tensor.matmul(ps2a, hs[:, sk, tt * P : (tt + 1) * P],
                                         sw2_sb[:, sk, 0:512],
                                         start=(sk == 0), stop=(sk == SHKO - 1))
                        nc.tensor.matmul(ps2b, hs[:, sk, tt * P : (tt + 1) * P],
                                         sw2_sb[:, sk, 512:1024],
                                         start=(sk == 0), stop=(sk == SHKO - 1))
                    nc.vector.tensor_copy(o_sb[:, 0:512], ps2a)
                    nc.vector.tensor_copy(o_sb[:, 512:1024], ps2b)
                    nc.sync.dma_start(shb[tt * P : (tt + 1) * P, :], o_sb)
                shrs = dram.tile([CH, DIM], bf16, tag=f"shrs{s}", name=f"shrs{s}")
                if sim:
                    nc.sync.dma_start(shrs[:, :], shb[0:CH, :])
                else:
                    nc.gpsimd.collective_compute(
                        "ReduceScatter", OP.add,
                        replica_groups=[list(range(NCORES))],
                        ins=[shb.opt()], outs=[shrs.opt()])
                nc.sync.dma_start(shout[s, :, :], shrs[:, :])

    nc.finalize()
    return nc


def _build_phase2(sim=False):
    import concourse.mybir as mybir
    import concourse.tile as tile
    from concourse import bacc
    from concourse.bass import IndirectOffsetOnAxis
    f32 = mybir.dt.float32
    bf16 = mybir.dt.bfloat16
    i32 = mybir.dt.int32
    AF = mybir.ActivationFunctionType
    OP = mybir.AluOpType
    nc = bacc.Bacc("TRN2", target_bir_lowering=False, debug=False,
                   num_devices=1 if sim else NCORES)

    xg = nc.dram_tensor("xg", [P, DKO, C], bf16, kind="ExternalInput")
    w1p = nc.dram_tensor("w1p", [HKO, P, DKO, P], bf16, kind="ExternalInput")
    b1c = nc.dram_tensor("b1c", [P, HKO], f32, kind="ExternalInput")
    w2p = nc.dram_tensor("w2p", [P, HKO, DIM], bf16, kind="ExternalInput")
    b2r = nc.dram_tensor("b2r", [1, DIM], f32, kind="ExternalInput")
    ceg = nc.dram_tensor("ceg", [P, CT], f32, kind="ExternalInput")
    idxs = [nc.dram_tensor(f"idx{c}", [P, CT], i32, kind="ExternalInput")
            for c in range(3)]
    EOUT = sum(r // NCORES for r in CHUNK_ROWS)
    eout = nc.dram_tensor("eout", [EOUT, DIM], bf16, kind="ExternalOutput")

    with tile.TileContext(nc) as tc:
        with (
            tc.tile_pool(name="const", bufs=1) as const,
            tc.tile_pool(name="wpool", bufs=1) as wpool,
            tc.tile_pool(name="hp", bufs=1) as hp,
            tc.tile_pool(name="ysb", bufs=2) as ysb,
            tc.tile_pool(name="dram", bufs=1, space="DRAM") as dram,
            tc.tile_pool(name="p1", bufs=3, space="PSUM") as p1p,
            tc.tile_pool(name="p2", bufs=2, space="PSUM") as p2p,
        ):
            b1c_sb = const.tile([P, HKO], f32)
            nc.sync.dma_start(b1c_sb, b1c[:, :])
            b2r_sb = const.tile([1, DIM], f32)
            nc.sync.dma_start(b2r_sb, b2r[:, :])
            ceg_sb = const.tile([P, CT], f32)
            nc.sync.dma_start(ceg_sb, ceg[:, :])
            idx_sbs = []
            for c in range(3):
                idx_sb_c = const.tile([P, CT], i32, name=f"idx_sb{c}")
                nc.sync.dma_start(idx_sb_c, idxs[c][:, :])
                idx_sbs.append(idx_sb_c)
            ones1 = const.tile([1, P], f32)
            nc.vector.memset(ones1, 1.0)
            zero_sb = const.tile([P, DIM], bf16)
            nc.vector.memset(zero_sb, 0.0)

            xg_sb = wpool.tile([P, DKO, C], bf16)
            c0 = 0
            for Wx in SLICES2:
                nc.sync.dma_start(xg_sb[:, :, c0 : c0 + Wx], xg[:, :, c0 : c0 + Wx])
                c0 += Wx
            w1_sb = wpool.tile([P, DKO, H], bf16)
            for hm in range(HKO):
                nc.sync.dma_start(w1_sb[:, :, hm * P : (hm + 1) * P],
                                  w1p[hm, :, :, :])
            w2_sb = wpool.tile([P, HKO, DIM], bf16)
            nc.sync.dma_start(w2_sb, w2p[:, :, :])

            bufs = []
            for c in range(3):
                buf_c = dram.tile([CHUNK_ROWS[c], DIM], bf16, name=f"buf{c}",
                                  tag=f"buf{c}")
                for r0 in range(0, CHUNK_TOK[c], P):
                    nc.sync.dma_start(buf_c[r0 : r0 + P, :], zero_sb)
                bufs.append(buf_c)

            sl0 = 0
            for s, W in enumerate(SLICES2):
                h = hp.tile([P, HKO, 512], bf16, tag="h")
                for hm in range(HKO):
                    ps = p1p.tile([P, 512], f32, tag="ps1")
                    for ko in range(DKO):
                        nc.tensor.matmul(ps[:, :W], w1_sb[:, ko, hm * P : (hm + 1) * P],
                                         xg_sb[:, ko, sl0 : sl0 + W],
                                         start=(ko == 0), stop=(ko == DKO - 1))
                    nc.scalar.activation(h[:, hm, :W], ps[:, :W], AF.Gelu,
                                         bias=b1c_sb[:, hm : hm + 1])
                for tt in range(W // P):
                    gtt = sl0 // P + tt
                    y_sb = ysb.tile([P, DIM], bf16, tag="y_sb")
                    ps2a = p2p.tile([P, 512], f32, tag="ps2_0")
                    ps2b = p2p.tile([P, 512], f32, tag="ps2_1")
                    for hk in range(HKO):
                        nc.tensor.matmul(ps2a, h[:, hk, tt * P : (tt + 1) * P],
                                         w2_sb[:, hk, 0:512],
                                         start=(hk == 0), stop=False)
                        nc.tensor.matmul(ps2b, h[:, hk, tt * P : (tt + 1) * P],
                                         w2_sb[:, hk, 512:1024],
                                         start=(hk == 0), stop=False)
                    nc.tensor.matmul(ps2a, ones1[0:1, 0:P], b2r_sb[0:1, 0:512],
                                     start=False, stop=True)
                    nc.tensor.matmul(ps2b, ones1[0:1, 0:P], b2r_sb[0:1, 512:1024],
                                     start=False, stop=True)
                    nc.vector.tensor_tensor(
                        y_sb[:, 0:512], ps2a,
                        ceg_sb[:, gtt : gtt + 1].to_broadcast((P, 512)), OP.mult)
                    nc.vector.tensor_tensor(
                        y_sb[:, 512:1024], ps2b,
                        ceg_sb[:, gtt : gtt + 1].to_broadcast((P, 512)), OP.mult)
                    for c in GROUP_CHUNKS[gtt]:
                        nc.gpsimd.indirect_dma_start(
                            out=bufs[c][:, :],
                            out_offset=IndirectOffsetOnAxis(
                                ap=idx_sbs[c][:, gtt : gtt + 1], axis=0),
                            in_=y_sb[:, :],
                            in_offset=None)
                sl0 += W
                # chunk s's writers are all in slices <= s: reduce it now so
                # the collective overlaps the remaining slices' compute
                csz = CHUNK_ROWS[s] // NCORES
                rs_c = dram.tile([csz, DIM], bf16, name=f"rs{s}", tag=f"rs{s}")
                if sim:
                    nc.sync.dma_start(rs_c[:, :], bufs[s][0:csz, :])
                else:
                    nc.gpsimd.collective_compute(
                        "ReduceScatter", OP.add,
                        replica_groups=[list(range(NCORES))],
                        ins=[bufs[s].opt()], outs=[rs_c.opt()])
                e0 = sum(r // NCORES for r in CHUNK_ROWS[:s])
                nc.sync.dma_start(eout[e0 : e0 + csz, :], rs_c[:, :])

    nc.finalize()
    return nc


def _get(name, builder):
    if name not in _nc_cache:
        _nc_cache[name] = builder()
    return _nc_cache[name]


def _prep_phase1(x, router_w, router_b, sw1, sb1, sw2, sb2):
    xt = np.ascontiguousarray(x.reshape(T, DIM).astype(np.float32).T)
    xt32p = np.ascontiguousarray(xt.reshape(DKO, P, T).transpose(1, 0, 2))
    rwp = np.ascontiguousarray(router_w.astype(np.float32).reshape(DKO, P, E).transpose(1, 0, 2))
    rb = np.ascontiguousarray(np.tile(router_b.astype(np.float32)[None, :], (P, 1)))
    maps = []
    for e in range(NCORES):
        s0 = e * SH
        sw1pe = np.ascontiguousarray(sw1[:, s0:s0 + SH].reshape(DKO, P, SH).transpose(1, 0, 2)).astype(BF16)
        sb1ce = np.ascontiguousarray(sb1[s0:s0 + SH].astype(np.float32).reshape(SHKO, P).T)
        sw2pe = np.ascontiguousarray(sw2[s0:s0 + SH, :].reshape(SHKO, P, DIM).transpose(1, 0, 2)).astype(BF16)
        maps.append(dict(xt32=xt32p, rwp=rwp, rb=rb, sw1p=sw1pe,
                         sb1c=sb1ce, sw2p=sw2pe))
    return maps, xt


def _prep_phase2(ct, xt, w1, b1, w2, b2):
    maps = []
    for e in range(NCORES):
        sel = np.nonzero(ct[:, e])[0].astype(np.int64)
        if len(sel) > C:
            return None  # over capacity: caller falls back to dense
        npad = C - len(sel)
        selp = np.concatenate([sel, np.zeros(npad, np.int64)])
        nreal = len(sel)
        cev = np.concatenate([ct[sel, e].astype(np.float32), np.zeros(npad, np.float32)])
        # verify the static group->chunk assignment holds for this routing
        slot_chunk = np.digitize(sel, CHUNK_BASE[1:])  # chunk id per real slot
        for g in range(CT):
            lo, hi = g * P, min((g + 1) * P, nreal)
            if lo >= hi:
                continue
            if not set(np.unique(slot_chunk[lo:hi])) <= set(GROUP_CHUNKS[g]):
                return None  # unexpected routing shape: dense fallback
        # per-chunk destination rows (pads and other-chunk slots -> dump rows)
        idxps = []
        for c in range(3):
            dump = CHUNK_TOK[c] + (np.arange(C) % 8)
            inchunk = np.zeros(C, bool)
            inchunk[:nreal] = slot_chunk == c
            dest = np.where(inchunk, selp - CHUNK_BASE[c], dump)
            idxps.append(np.ascontiguousarray(dest.reshape(CT, P).T.astype(np.int32)))
        xge = xt[:, selp]  # [DIM, C] f32
        xgp = np.ascontiguousarray(xge.reshape(DKO, P, C).transpose(1, 0, 2)).astype(BF16)
        w1pe = np.ascontiguousarray(
            w1[e].reshape(DKO, P, HKO, P).transpose(2, 1, 0, 3)).astype(BF16)
        b1ce = np.ascontiguousarray(b1[e].astype(np.float32).reshape(HKO, P).T)
        w2pe = np.ascontiguousarray(w2[e].reshape(HKO, P, DIM).transpose(1, 0, 2)).astype(BF16)
        cegp = np.ascontiguousarray(cev.reshape(CT, P).T)
        maps.append(dict(xg=xgp, w1p=w1pe, b1c=b1ce, w2p=w2pe,
                         b2r=np.ascontiguousarray(b2[e].astype(np.float32)[None, :]),
                         ceg=cegp, idx0=idxps[0], idx1=idxps[1], idx2=idxps[2]))
    return maps


def _run_spmd(nc, in_maps, trace=False):
    from concourse.bass_utils import run_bass_kernel_spmd
    return run_bass_kernel_spmd(nc, in_maps, core_ids=list(range(NCORES)), trace=trace)


def _kernel_dense(x, router_w, router_b, w1, b1, w2, b2, sw1, sb1, sw2, sb2):
    in_maps = _prep_in_maps(x, router_w, router_b, w1, b1, w2, b2, sw1, sb1, sw2, sb2)
    res = _run_spmd(_get("dense", _build_nc), in_maps)
    outT = np.concatenate([r["out"] for r in res.results], axis=0)
    return np.ascontiguousarray(outT.T).reshape(2, 2048, DIM)


def _kernel_sparse(x, router_w, router_b, w1, b1, w2, b2, sw1, sb1, sw2, sb2):
    maps1, xt = _prep_phase1(x, router_w, router_b, sw1, sb1, sw2, sb2)
    res1 = _run_spmd(_get("p1", _build_phase1), maps1)
    ct = res1.results[0]["ct"]  # [T, E] combine weights (same on all cores)

    CH = T // NSLICES // NCORES
    shared = np.zeros((T, DIM), np.float32)
    for j in range(NCORES):
        sh = res1.results[j]["shout"].astype(np.float32)  # [NSLICES, CH, DIM]
        for s in range(NSLICES):
            r0 = s * SLICE + j * CH
            shared[r0:r0 + CH] = sh[s]
    shared += sb2.astype(np.float32)[None, :]
    maps2 = _prep_phase2(ct, xt, w1, b1, w2, b2)
    if maps2 is None:
        return _kernel_dense(x, router_w, router_b, w1, b1, w2, b2,
                             sw1, sb1, sw2, sb2)
    res2 = _run_spmd(_get("p2", _build_phase2), maps2)
    expert = np.empty((T, DIM), np.float32)
    e0 = 0
    for c in range(3):
        csz = CHUNK_ROWS[c] // NCORES
        seg = np.concatenate(
            [r["eout"][e0 : e0 + csz] for r in res2.results], axis=0)
        expert[CHUNK_BASE[c] : CHUNK_BASE[c] + CHUNK_TOK[c]] = seg[: CHUNK_TOK[c]]
        e0 += csz
    return (expert + shared).reshape(2, 2048, DIM)


def kernel(x, router_w, router_b, w1, b1, w2, b2, sw1, sb1, sw2, sb2):
    import os
    if os.environ.get("MOE_DENSE"):
        return _kernel_dense(x, router_w, router_b, w1, b1, w2, b2, sw1, sb1, sw2, sb2)
    return _kernel_sparse(x, router_w, router_b, w1, b1, w2, b2, sw1, sb1, sw2, sb2)



# revision 11
# speedup vs baseline: 1.5540x; 1.5540x over previous
"""Trainium2 Bass kernel for an 8-expert top-2 MoE layer (+ shared expert).

Expert-parallel, two-NEFF design over 8 NeuronCores, with all large GEMMs
run as fp8(e4m3) DoubleRow matmuls using a 3-term residual decomposition:

    a @ b  ~=  a_hi @ b_hi + a_lo @ b_hi + a_hi @ b_lo

where `_hi = e4m3(v)` and `_lo = e4m3(v - _hi)` (same scale, so all three
terms accumulate in one fp32 PSUM group). DoubleRow mode contracts two
128-deep k-tiles per instruction, pairing same-k (hi,hi), (lo,hi), (hi,lo)
plane tiles, so no operand duplication is needed. This is both faster than
bf16 (12 half-rate groups vs 16 full-rate per K=1024 pair of planes) and
more accurate (measured 2.1e-3 max-rel vs 5.5e-3 for the bf16 baseline).

Phase 1 (one NEFF, SPMD): token-parallel. Core j owns tokens
[j*512,(j+1)*512): replicated fp32 router (exact top-2 selection from fp32
logits, matching the reference), plus the full shared-expert FFN for its
own tokens. No collectives: each core returns its combine-weight slice and
its shared-output slice to the host.

Host dispatch: builds per-expert gather lists from ct (the all-to-all by
top-k expert id), pads to capacity C=1152, gathers pre-split fp8 hi/lo
activations per core. Any routing overflow beyond C is computed on the
host in fp32 (never triggers for the fixed benchmark input).

Phase 2 (one NEFF, SPMD): expert-parallel. Core e runs expert e's FFN over
its <=C routed tokens (fp8 residual matmuls, fp32 PSUM, erf-Gelu), applies
the combine weight on-device and returns dense slot-ordered fp32 outputs.
No collectives: the host scatter-adds the two expert contributions per
token onto the shared output (the unshard step).

Everything computes in the transposed-activation layout (tokens on the
free axis) so no on-device transposes are needed.
"""

import sys

if "/opt/trn_rl_repo" not in sys.path:
    sys.path.insert(0, "/opt/trn_rl_repo")

import math

import numpy as np
import ml_dtypes

DIM = 1024
E = 8
H = 4096
T = 4096  # B*S = 2*2048 tokens
NCORES = 8
P = 128
DKO = DIM // P     # 8 k-subtiles over dim
HKO = H // P       # 32 k-subtiles over hidden
TS = T // NCORES   # 512 tokens per core in phase 1
NT4 = TS // P      # 4 token tiles per phase-1 slice

C = 1152           # per-expert token capacity (actual max 1091)
SLICES2 = [512, 512, 128]  # phase-2 token slices (sum = C)

E4M3 = ml_dtypes.float8_e4m3
F32 = np.float32

_nc_cache = {}


def _split_fp8(a):
    """e4m3 hi/lo residual split (same scale for both planes)."""
    hi = np.asarray(a, E4M3)
    lo = np.asarray(a - hi.astype(F32), E4M3)
    return hi, lo


def _quant_w(w):
    """Scale so absmax lands in (112, 224], split hi/lo. Returns
    ([K/128, two, ...] packed [P, 2, KO, N] planes, inv_scale)."""
    m = float(np.abs(w).max())
    s = 2.0 ** math.floor(math.log2(224.0 / m)) if m > 0 else 1.0
    hi, lo = _split_fp8(w.astype(F32) * s)
    K, N = w.shape
    ko = K // P
    pack = np.empty((P, 2, ko, N), E4M3)
    pack[:, 0] = hi.reshape(ko, P, N).transpose(1, 0, 2)
    pack[:, 1] = lo.reshape(ko, P, N).transpose(1, 0, 2)
    return np.ascontiguousarray(pack), 1.0 / s


def _dr_steps(nc, PM, ps, wt, m0, m1, xt_, n0, n1, ko):
    """Emit the 3-term fp8 DoubleRow accumulation over `ko` k-tiles.

    wt: [P, 2, ko, M] tile (cols m0:m1), xt_: [P, 2, ko, N] tile (cols
    n0:n1), ps: [m1-m0, n1-n0] PSUM. Pairs of k-tiles are contracted per
    instruction; terms (hi,hi), (lo,hi), (hi,lo) share one PSUM scale.
    """
    steps = []
    for k0 in range(0, ko, 2):
        steps.append(((0, k0), (0, k0)))
        steps.append(((1, k0), (0, k0)))
        steps.append(((0, k0), (1, k0)))
    for i, ((wp, wk), (xp, xk)) in enumerate(steps):
        nc.tensor.matmul(ps,
                         wt[:, wp, wk:wk + 2, m0:m1],
                         xt_[:, xp, xk:xk + 2, n0:n1],
                         start=(i == 0),
                         stop=(i == len(steps) - 1), perf_mode=PM.DoubleRow)


def _build_phase1():
    import concourse.mybir as mybir
    import concourse.tile as tile
    from concourse import bacc

    f32 = mybir.dt.float32
    fp8 = mybir.dt.float8e4
    AF = mybir.ActivationFunctionType
    OP = mybir.AluOpType
    AX = mybir.AxisListType
    PM = mybir.MatmulPerfMode

    nc = bacc.Bacc("TRN2", target_bir_lowering=False, debug=False,
                   num_devices=NCORES)

    xs32 = nc.dram_tensor("xs32", [P, DKO, TS], f32, kind="ExternalInput")
    xs8 = nc.dram_tensor("xs8", [P, 2, DKO, TS], fp8, kind="ExternalInput")
    rwp = nc.dram_tensor("rwp", [P, DKO, E], f32, kind="ExternalInput")
    rb = nc.dram_tensor("rb", [P, E], f32, kind="ExternalInput")
    sw1q = nc.dram_tensor("sw1q", [P, 2, DKO, H], fp8, kind="ExternalInput")
    sb1c = nc.dram_tensor("sb1c", [P, HKO], f32, kind="ExternalInput")
    sw2q = nc.dram_tensor("sw2q", [P, 2, HKO, DIM], fp8, kind="ExternalInput")
    sb2c = nc.dram_tensor("sb2c", [P, DKO], f32, kind="ExternalInput")
    scal = nc.dram_tensor("scal", [P, 2], f32, kind="ExternalInput")
    ct = nc.dram_tensor("ct", [TS, E], f32, kind="ExternalOutput")
    sh = nc.dram_tensor("sh", [P, DKO, TS], f32, kind="ExternalOutput")

    with tile.TileContext(nc) as tc:
        with (
            tc.tile_pool(name="const", bufs=1) as const,
            tc.tile_pool(name="wpool", bufs=1) as wpool,
            tc.tile_pool(name="rt", bufs=2) as rt,
            tc.tile_pool(name="gp", bufs=3) as gp,
            tc.tile_pool(name="hp", bufs=1) as hp,
            tc.tile_pool(name="op", bufs=2) as op_,
            tc.tile_pool(name="rps", bufs=2, space="PSUM") as rps,
            tc.tile_pool(name="p1", bufs=3, space="PSUM") as p1p,
            tc.tile_pool(name="p2", bufs=2, space="PSUM") as p2p,
        ):
            # DMA issue order: what the first matmuls need comes first.
            sb1c_sb = const.tile([P, HKO], f32)
            nc.sync.dma_start(sb1c_sb, sb1c[:, :])
            scal_sb = const.tile([P, 2], f32)
            nc.sync.dma_start(scal_sb, scal[:, :])
            x8_sb = wpool.tile([P, 2, DKO, TS], fp8)
            nc.sync.dma_start(x8_sb, xs8[:, :, :, :])
            # chunked weight loads so the first matmuls can start early
            sw1_sb = wpool.tile([P, 2, DKO, H], fp8)
            for h0 in range(0, 1024, 512):
                nc.sync.dma_start(sw1_sb[:, :, :, h0:h0 + 512],
                                  sw1q[:, :, :, h0:h0 + 512])
            x32_sb = wpool.tile([P, DKO, TS], f32)
            nc.sync.dma_start(x32_sb, xs32[:, :, :])
            for h0 in range(1024, H, 512):
                nc.sync.dma_start(sw1_sb[:, :, :, h0:h0 + 512],
                                  sw1q[:, :, :, h0:h0 + 512])
            rwp_sb = const.tile([P, DKO, E], f32)
            nc.sync.dma_start(rwp_sb, rwp[:, :, :])
            rb_sb = const.tile([P, E], f32)
            nc.sync.dma_start(rb_sb, rb[:, :])
            sb2c_sb = const.tile([P, DKO], f32)
            nc.sync.dma_start(sb2c_sb, sb2c[:, :])
            sw2_sb = wpool.tile([P, 2, HKO, DIM], fp8)
            for d0 in range(0, DIM, 512):
                nc.sync.dma_start(sw2_sb[:, :, :, d0:d0 + 512],
                                  sw2q[:, :, :, d0:d0 + 512])

            # ---- shared expert FFN (layer 1) on this core's 512 tokens ----
            h8 = hp.tile([P, 2, HKO, TS], fp8, tag="h8")
            for hm in range(HKO):
                ps = p1p.tile([P, TS], f32, tag="ps1")
                _dr_steps(nc, PM, ps, sw1_sb, hm * P, (hm + 1) * P,
                          x8_sb, 0, TS, DKO)
                g = gp.tile([P, TS], f32, tag="g")
                nc.scalar.activation(g, ps, AF.Gelu,
                                     bias=sb1c_sb[:, hm:hm + 1],
                                     scale=scal_sb[:, 0:1])
                nc.gpsimd.tensor_copy(h8[:, 0, hm, :], g)
                nc.vector.scalar_tensor_tensor(
                    h8[:, 1, hm, :], g, 1.0, h8[:, 0, hm, :],
                    OP.mult, OP.subtract)

            # ---- router (fills the PE bubble between the two layers):
            # fp32 logits, softmax, exact top-2 (batched) ----
            lg4 = rt.tile([P, NT4, E], f32, tag="lg4")
            for t4 in range(NT4):
                pl = rps.tile([P, E], f32, tag="pl")
                for ko in range(DKO):
                    nc.tensor.matmul(pl, x32_sb[:, ko, t4 * P:(t4 + 1) * P],
                                     rwp_sb[:, ko, :],
                                     start=(ko == 0), stop=(ko == DKO - 1))
                nc.vector.tensor_add(lg4[:, t4, :], pl, rb_sb)
            mx4 = rt.tile([P, NT4, 1], f32, tag="mx4")
            nc.vector.reduce_max(mx4, lg4, axis=AX.X)
            lgs = rt.tile([P, NT4, E], f32, tag="lgs")
            nc.vector.tensor_sub(lgs, lg4, mx4.to_broadcast((P, NT4, E)))
            ex4 = rt.tile([P, NT4, E], f32, tag="ex4")
            nc.scalar.activation(ex4, lgs, AF.Exp)
            sm4 = rt.tile([P, NT4, 1], f32, tag="sm4")
            nc.vector.reduce_sum(sm4, ex4, axis=AX.X)
            rc4 = rt.tile([P, NT4, 1], f32, tag="rc4")
            nc.vector.reciprocal(rc4, sm4)
            ge1 = rt.tile([P, NT4, E], f32, tag="ge1")
            nc.vector.tensor_scalar(ge1, lgs, 0.0, 1e30, OP.is_ge, OP.mult)
            lm4 = rt.tile([P, NT4, E], f32, tag="lm4")
            nc.vector.tensor_sub(lm4, lgs, ge1)
            m24 = rt.tile([P, NT4, 1], f32, tag="m24")
            nc.vector.reduce_max(m24, lm4, axis=AX.X)
            msk4 = rt.tile([P, NT4, E], f32, tag="msk4")
            nc.vector.tensor_tensor(msk4, lgs, m24.to_broadcast((P, NT4, E)),
                                    OP.is_ge)
            pw4 = rt.tile([P, NT4, E], f32, tag="pw4")
            nc.vector.tensor_mul(pw4, ex4, msk4)
            nc.vector.tensor_tensor(pw4, pw4, rc4.to_broadcast((P, NT4, E)),
                                    OP.mult)
            nc.sync.dma_start(
                ct[:, :].rearrange("(t4 p) e -> p t4 e", p=P), pw4)

            # ---- shared expert FFN (layer 2) ----
            for dm in range(DKO):
                ps2 = p2p.tile([P, TS], f32, tag="ps2")
                _dr_steps(nc, PM, ps2, sw2_sb, dm * P, (dm + 1) * P,
                          h8, 0, TS, HKO)
                o_sb = op_.tile([P, TS], f32, tag="o_sb")
                nc.scalar.activation(o_sb, ps2, AF.Identity,
                                     bias=sb2c_sb[:, dm:dm + 1],
                                     scale=scal_sb[:, 1:2])
                nc.sync.dma_start(sh[:, dm, :], o_sb)

    nc.finalize()
    return nc


def _build_phase2():
    import concourse.mybir as mybir
    import concourse.tile as tile
    from concourse import bacc

    f32 = mybir.dt.float32
    fp8 = mybir.dt.float8e4
    AF = mybir.ActivationFunctionType
    OP = mybir.AluOpType
    PM = mybir.MatmulPerfMode

    nc = bacc.Bacc("TRN2", target_bir_lowering=False, debug=False,
                   num_devices=NCORES)

    xg8 = nc.dram_tensor("xg8", [P, 2, DKO, C], fp8, kind="ExternalInput")
    w1q = nc.dram_tensor("w1q", [P, 2, DKO, H], fp8, kind="ExternalInput")
    b1c = nc.dram_tensor("b1c", [P, HKO], f32, kind="ExternalInput")
    w2q = nc.dram_tensor("w2q", [P, 2, HKO, DIM], fp8, kind="ExternalInput")
    b2c = nc.dram_tensor("b2c", [P, DKO], f32, kind="ExternalInput")
    ceg = nc.dram_tensor("ceg", [P, C], f32, kind="ExternalInput")
    scal = nc.dram_tensor("scal", [P, 2], f32, kind="ExternalInput")
    eo = nc.dram_tensor("eo", [P, DKO, C], f32, kind="ExternalOutput")

    with tile.TileContext(nc) as tc:
        with (
            tc.tile_pool(name="const", bufs=1) as const,
            tc.tile_pool(name="wpool", bufs=1) as wpool,
            tc.tile_pool(name="gp", bufs=3) as gp,
            tc.tile_pool(name="hp", bufs=1) as hp,
            tc.tile_pool(name="op", bufs=2) as op_,
            tc.tile_pool(name="p1", bufs=4, space="PSUM") as p1p,
            tc.tile_pool(name="p2", bufs=3, space="PSUM") as p2p,
        ):
            b1c_sb = const.tile([P, HKO], f32)
            nc.sync.dma_start(b1c_sb, b1c[:, :])
            scal_sb = const.tile([P, 2], f32)
            nc.sync.dma_start(scal_sb, scal[:, :])
            xg_sb = wpool.tile([P, 2, DKO, C], fp8)
            nc.sync.dma_start(xg_sb[:, :, :, 0:512], xg8[:, :, :, 0:512])
            w1_sb = wpool.tile([P, 2, DKO, H], fp8)
            nc.sync.dma_start(w1_sb[:, :, :, 0:512], w1q[:, :, :, 0:512])
            nc.sync.dma_start(xg_sb[:, :, :, 512:C], xg8[:, :, :, 512:C])
            for h0 in range(512, H, 512):
                nc.sync.dma_start(w1_sb[:, :, :, h0:h0 + 512],
                                  w1q[:, :, :, h0:h0 + 512])
            b2c_sb = const.tile([P, DKO], f32)
            nc.sync.dma_start(b2c_sb, b2c[:, :])
            ceg_sb = const.tile([P, C], f32)
            nc.sync.dma_start(ceg_sb, ceg[:, :])
            w2_sb = wpool.tile([P, 2, HKO, DIM], fp8)
            for d0 in range(0, DIM, 512):
                nc.sync.dma_start(w2_sb[:, :, :, d0:d0 + 512],
                                  w2q[:, :, :, d0:d0 + 512])

            c0 = 0
            for s, W in enumerate(SLICES2):
                h8 = hp.tile([P, 2, HKO, 512], fp8, tag="h8")
                for hm in range(HKO):
                    ps = p1p.tile([P, 512], f32, tag="ps1")
                    _dr_steps(nc, PM, ps[:, :W], w1_sb, hm * P,
                              (hm + 1) * P, xg_sb, c0, c0 + W, DKO)
                    g = gp.tile([P, 512], f32, tag="g")
                    nc.scalar.activation(g[:, :W], ps[:, :W], AF.Gelu,
                                         bias=b1c_sb[:, hm:hm + 1],
                                         scale=scal_sb[:, 0:1])
                    nc.gpsimd.tensor_copy(h8[:, 0, hm, :W], g[:, :W])
                    nc.vector.scalar_tensor_tensor(
                        h8[:, 1, hm, :W], g[:, :W], 1.0, h8[:, 0, hm, :W],
                        OP.mult, OP.subtract)
                for dm in range(DKO):
                    ps2 = p2p.tile([P, 512], f32, tag="ps2")
                    _dr_steps(nc, PM, ps2[:, :W], w2_sb, dm * P,
                              (dm + 1) * P, h8, 0, W, HKO)
                    o_sb = op_.tile([P, 512], f32, tag="o_sb")
                    nc.scalar.activation(o_sb[:, :W], ps2[:, :W], AF.Identity,
                                         bias=b2c_sb[:, dm:dm + 1],
                                         scale=scal_sb[:, 1:2])
                    nc.vector.tensor_mul(o_sb[:, :W], o_sb[:, :W],
                                         ceg_sb[:, c0:c0 + W])
                    nc.sync.dma_start(eo[:, dm, c0:c0 + W], o_sb[:, :W])
                c0 += W

    nc.finalize()
    return nc


def _get(name, builder):
    if name not in _nc_cache:
        _nc_cache[name] = builder()
    return _nc_cache[name]


def _run_spmd(nc, in_maps):
    from concourse.bass_utils import run_bass_kernel_spmd
    return run_bass_kernel_spmd(nc, in_maps, core_ids=list(range(NCORES)))


def _gelu_np(v):
    from scipy.special import erf
    return 0.5 * v * (1.0 + erf(v / np.sqrt(2.0)))


def kernel(x, router_w, router_b, w1, b1, w2, b2, sw1, sb1, sw2, sb2):
    x = np.asarray(x, F32)
    xt = np.ascontiguousarray(x.reshape(T, DIM).T)          # [DIM, T]
    xt32p = np.ascontiguousarray(
        xt.reshape(DKO, P, T).transpose(1, 0, 2))           # [P, DKO, T]
    xhi, xlo = _split_fp8(xt)
    x8p = np.empty((P, 2, DKO, T), E4M3)
    x8p[:, 0] = xhi.reshape(DKO, P, T).transpose(1, 0, 2)
    x8p[:, 1] = xlo.reshape(DKO, P, T).transpose(1, 0, 2)

    rwp = np.ascontiguousarray(
        np.asarray(router_w, F32).reshape(DKO, P, E).transpose(1, 0, 2))
    rbp = np.ascontiguousarray(
        np.tile(np.asarray(router_b, F32)[None, :], (P, 1)))

    sw1q, s1 = _quant_w(np.asarray(sw1, F32))
    sw2q, s2 = _quant_w(np.asarray(sw2, F32))
    sb1cp = np.ascontiguousarray(np.asarray(sb1, F32).reshape(HKO, P).T)
    sb2cp = np.ascontiguousarray(np.asarray(sb2, F32).reshape(DKO, P).T)
    sscal = np.ascontiguousarray(np.tile(np.array([[s1, s2]], F32), (P, 1)))

    maps1 = []
    for j in range(NCORES):
        t0 = j * TS
        maps1.append(dict(
            xs32=np.ascontiguousarray(xt32p[:, :, t0:t0 + TS]),
            xs8=np.ascontiguousarray(x8p[:, :, :, t0:t0 + TS]),
            rwp=rwp, rb=rbp, sw1q=sw1q, sb1c=sb1cp, sw2q=sw2q,
            sb2c=sb2cp, scal=sscal))
    res1 = _run_spmd(_get("p1", _build_phase1), maps1)

    ctf = np.concatenate([r["ct"] for r in res1.results], axis=0)  # [T, E]
    shared = np.empty((T, DIM), F32)
    for j, r in enumerate(res1.results):
        # sh [P, DKO, TS]: value (ki, dm, t) = shared[dim dm*128+ki, tok]
        shj = r["sh"].transpose(2, 1, 0).reshape(TS, DIM)
        shared[j * TS:(j + 1) * TS] = shj

    # ---- host all-to-all dispatch by top-k expert id ----
    maps2 = []
    sels = []
    overflow = []  # (expert, token ids) computed on host in fp32
    for e in range(NCORES):
        sel = np.nonzero(ctf[:, e])[0]
        if len(sel) > C:
            overflow.append((e, sel[C:]))
            sel = sel[:C]
        sels.append(sel)
        npad = C - len(sel)
        selp = np.concatenate([sel, np.zeros(npad, np.int64)])
        cev = np.concatenate([ctf[sel, e].astype(F32), np.zeros(npad, F32)])
        w1qe, e1 = _quant_w(np.asarray(w1[e], F32))
        w2qe, e2 = _quant_w(np.asarray(w2[e], F32))
        maps2.append(dict(
            xg8=np.ascontiguousarray(x8p[:, :, :, selp]),
            w1q=w1qe,
            b1c=np.ascontiguousarray(np.asarray(b1[e], F32).reshape(HKO, P).T),
            w2q=w2qe,
            b2c=np.ascontiguousarray(np.asarray(b2[e], F32).reshape(DKO, P).T),
            ceg=np.ascontiguousarray(np.tile(cev[None, :], (P, 1))),
            scal=np.ascontiguousarray(np.tile(np.array([[e1, e2]], F32), (P, 1)))))
    res2 = _run_spmd(_get("p2", _build_phase2), maps2)

    out = shared
    for e, r in enumerate(res2.results):
        n = len(sels[e])
        # eo [P, DKO, C] -> token-major [C, DIM]
        ye = r["eo"].transpose(2, 1, 0).reshape(C, DIM)
        out[sels[e]] += ye[:n]
    for e, toks in overflow:
        xe = x.reshape(T, DIM)[toks]
        he = _gelu_np(xe @ np.asarray(w1[e], F32) + np.asarray(b1[e], F32))
        ye = he @ np.asarray(w2[e], F32) + np.asarray(b2[e], F32)
        out[toks] += ye * ctf[toks, e:e + 1]

    return out.reshape(2, 2048, DIM)


# revision 16
# speedup vs baseline: 1.6257x; 1.0461x over previous
"""Trainium2 Bass kernel for an 8-expert top-2 MoE layer (+ shared expert).

Expert-parallel, two-NEFF design over 8 NeuronCores, with all large GEMMs
run as fp8(e4m3) DoubleRow matmuls using a 3-term residual decomposition:

    a @ b  ~=  a_hi @ b_hi + a_lo @ b_hi + a_hi @ b_lo

where `_hi = e4m3(v)` and `_lo = e4m3(v - _hi)` (same scale, so all three
terms accumulate in one fp32 PSUM group). DoubleRow mode contracts two
128-deep k-tiles per instruction, pairing same-k (hi,hi), (lo,hi), (hi,lo)
plane tiles, so no operand duplication is needed. This is both faster than
bf16 (12 half-rate groups vs 16 full-rate per K=1024 pair of planes) and
more accurate (measured 2.1e-3 max-rel vs 5.5e-3 for the bf16 baseline).

Phase 1 (one NEFF, SPMD): token-parallel. Core j owns tokens
[j*512,(j+1)*512): replicated fp32 router (exact top-2 selection from fp32
logits, matching the reference), plus the full shared-expert FFN for its
own tokens. No collectives: each core returns its combine-weight slice and
its shared-output slice to the host.

Host dispatch: builds per-expert gather lists from ct (the all-to-all by
top-k expert id), pads to capacity C=1152, gathers pre-split fp8 hi/lo
activations per core. Any routing overflow beyond C is computed on the
host in fp32 (never triggers for the fixed benchmark input).

Phase 2 (one NEFF, SPMD): expert-parallel. Core e runs expert e's FFN over
its <=C routed tokens (fp8 residual matmuls, fp32 PSUM, erf-Gelu), applies
the combine weight on-device and returns dense slot-ordered fp32 outputs.
No collectives: the host scatter-adds the two expert contributions per
token onto the shared output (the unshard step).

Everything computes in the transposed-activation layout (tokens on the
free axis) so no on-device transposes are needed.
"""

import sys

if "/opt/trn_rl_repo" not in sys.path:
    sys.path.insert(0, "/opt/trn_rl_repo")

import math

import numpy as np
import ml_dtypes

DIM = 1024
E = 8
H = 4096
T = 4096  # B*S = 2*2048 tokens
NCORES = 8
P = 128
DKO = DIM // P     # 8 k-subtiles over dim
HKO = H // P       # 32 k-subtiles over hidden
TS = T // NCORES   # 512 tokens per core in phase 1
NT4 = TS // P      # 4 token tiles per phase-1 slice

CAP_QUANT = 32     # round per-expert capacity up to a multiple of this


def _slices_for(cap):
    out = []
    while cap > 0:
        w = min(512, cap)
        out.append(w)
        cap -= w
    return out

E4M3 = ml_dtypes.float8_e4m3
F32 = np.float32

_nc_cache = {}


def _split_fp8(a):
    """e4m3 hi/lo residual split (same scale for both planes)."""
    hi = np.asarray(a, E4M3)
    lo = np.asarray(a - hi.astype(F32), E4M3)
    return hi, lo


def _quant_w(w):
    """Scale so absmax lands in (112, 224], split hi/lo. Returns
    ([K/128, two, ...] packed [P, 2, KO, N] planes, inv_scale)."""
    m = float(np.abs(w).max())
    s = 2.0 ** math.floor(math.log2(224.0 / m)) if m > 0 else 1.0
    hi, lo = _split_fp8(w.astype(F32) * s)
    K, N = w.shape
    ko = K // P
    pack = np.empty((P, 2, ko, N), E4M3)
    pack[:, 0] = hi.reshape(ko, P, N).transpose(1, 0, 2)
    pack[:, 1] = lo.reshape(ko, P, N).transpose(1, 0, 2)
    return np.ascontiguousarray(pack), 1.0 / s


def _dr_steps(nc, PM, ps, wt, m0, m1, xt_, n0, n1, ko):
    """Emit the 3-term fp8 DoubleRow accumulation over `ko` k-tiles.

    wt: [P, 2, ko, M] tile (cols m0:m1), xt_: [P, 2, ko, N] tile (cols
    n0:n1), ps: [m1-m0, n1-n0] PSUM. Pairs of k-tiles are contracted per
    instruction; terms (hi,hi), (lo,hi), (hi,lo) share one PSUM scale.
    """
    steps = []
    for k0 in range(0, ko, 2):
        steps.append(((0, k0), (0, k0)))
        steps.append(((1, k0), (0, k0)))
        steps.append(((0, k0), (1, k0)))
    for i, ((wp, wk), (xp, xk)) in enumerate(steps):
        nc.tensor.matmul(ps,
                         wt[:, wp, wk:wk + 2, m0:m1],
                         xt_[:, xp, xk:xk + 2, n0:n1],
                         start=(i == 0),
                         stop=(i == len(steps) - 1), perf_mode=PM.DoubleRow)


def _build_phase1():
    import concourse.mybir as mybir
    import concourse.tile as tile
    from concourse import bacc

    f32 = mybir.dt.float32
    fp8 = mybir.dt.float8e4
    AF = mybir.ActivationFunctionType
    OP = mybir.AluOpType
    AX = mybir.AxisListType
    PM = mybir.MatmulPerfMode

    nc = bacc.Bacc("TRN2", target_bir_lowering=False, debug=False,
                   num_devices=NCORES)

    xs32 = nc.dram_tensor("xs32", [P, DKO, TS], f32, kind="ExternalInput")
    xs8 = nc.dram_tensor("xs8", [P, 2, DKO, TS], fp8, kind="ExternalInput")
    rwp = nc.dram_tensor("rwp", [P, DKO, E], f32, kind="ExternalInput")
    rb = nc.dram_tensor("rb", [P, E], f32, kind="ExternalInput")
    sw1q = nc.dram_tensor("sw1q", [P, 2, DKO, H], fp8, kind="ExternalInput")
    sb1c = nc.dram_tensor("sb1c", [P, HKO], f32, kind="ExternalInput")
    sw2q = nc.dram_tensor("sw2q", [P, 2, HKO, DIM], fp8, kind="ExternalInput")
    sb2c = nc.dram_tensor("sb2c", [P, DKO], f32, kind="ExternalInput")
    scal = nc.dram_tensor("scal", [P, 2], f32, kind="ExternalInput")
    ct = nc.dram_tensor("ct", [TS, E], f32, kind="ExternalOutput")
    sh = nc.dram_tensor("sh", [P, DKO, TS], f32, kind="ExternalOutput")

    with tile.TileContext(nc) as tc:
        with (
            tc.tile_pool(name="const", bufs=1) as const,
            tc.tile_pool(name="wpool", bufs=1) as wpool,
            tc.tile_pool(name="rt", bufs=2) as rt,
            tc.tile_pool(name="gp", bufs=3) as gp,
            tc.tile_pool(name="hp", bufs=1) as hp,
            tc.tile_pool(name="op", bufs=2) as op_,
            tc.tile_pool(name="rps", bufs=2, space="PSUM") as rps,
            tc.tile_pool(name="p1", bufs=3, space="PSUM") as p1p,
            tc.tile_pool(name="p2", bufs=2, space="PSUM") as p2p,
        ):
            # DMA issue order: what the first matmuls need comes first.
            x8_sb = wpool.tile([P, 2, DKO, TS], fp8)
            nc.sync.dma_start(x8_sb, xs8[:, :, :, :])
            # chunked weight loads so the first matmuls can start early
            sw1_sb = wpool.tile([P, 2, DKO, H], fp8)
            nc.sync.dma_start(sw1_sb[:, :, :, 0:128], sw1q[:, :, :, 0:128])
            sb1c_sb = const.tile([P, HKO], f32)
            nc.sync.dma_start(sb1c_sb, sb1c[:, :])
            scal_sb = const.tile([P, 2], f32)
            nc.sync.dma_start(scal_sb, scal[:, :])
            for h0 in range(128, H, 512):
                h1 = min(h0 + 512, H)
                nc.sync.dma_start(sw1_sb[:, :, :, h0:h1],
                                  sw1q[:, :, :, h0:h1])
            x32_sb = wpool.tile([P, DKO, TS], f32)
            nc.sync.dma_start(x32_sb, xs32[:, :, :])
            rwp_sb = const.tile([P, DKO, E], f32)
            nc.sync.dma_start(rwp_sb, rwp[:, :, :])
            rb_sb = const.tile([P, E], f32)
            nc.sync.dma_start(rb_sb, rb[:, :])
            sb2c_sb = const.tile([P, DKO], f32)
            nc.sync.dma_start(sb2c_sb, sb2c[:, :])
            sw2_sb = wpool.tile([P, 2, HKO, DIM], fp8)
            for d0 in range(0, DIM, 512):
                nc.sync.dma_start(sw2_sb[:, :, :, d0:d0 + 512],
                                  sw2q[:, :, :, d0:d0 + 512])

            # ---- shared expert FFN (layer 1) on this core's 512 tokens ----
            h8 = hp.tile([P, 2, HKO, TS], fp8, tag="h8")
            for hm in range(HKO):
                ps = p1p.tile([P, TS], f32, tag="ps1")
                _dr_steps(nc, PM, ps, sw1_sb, hm * P, (hm + 1) * P,
                          x8_sb, 0, TS, DKO)
                g = gp.tile([P, TS], f32, tag="g")
                nc.scalar.activation(g, ps, AF.Gelu,
                                     bias=sb1c_sb[:, hm:hm + 1],
                                     scale=scal_sb[:, 0:1])
                nc.gpsimd.tensor_copy(h8[:, 0, hm, :], g)
                nc.vector.scalar_tensor_tensor(
                    h8[:, 1, hm, :], g, 1.0, h8[:, 0, hm, :],
                    OP.mult, OP.subtract)

            # ---- router (fills the PE bubble between the two layers):
            # fp32 logits, softmax, exact top-2 (batched) ----
            lg4 = rt.tile([P, NT4, E], f32, tag="lg4")
            for t4 in range(NT4):
                pl = rps.tile([P, E], f32, tag="pl")
                for ko in range(DKO):
                    nc.tensor.matmul(pl, x32_sb[:, ko, t4 * P:(t4 + 1) * P],
                                     rwp_sb[:, ko, :],
                                     start=(ko == 0), stop=(ko == DKO - 1))
                nc.vector.tensor_add(lg4[:, t4, :], pl, rb_sb)
            mx4 = rt.tile([P, NT4, 1], f32, tag="mx4")
            nc.vector.reduce_max(mx4, lg4, axis=AX.X)
            lgs = rt.tile([P, NT4, E], f32, tag="lgs")
            nc.vector.tensor_sub(lgs, lg4, mx4.to_broadcast((P, NT4, E)))
            ex4 = rt.tile([P, NT4, E], f32, tag="ex4")
            nc.scalar.activation(ex4, lgs, AF.Exp)
            sm4 = rt.tile([P, NT4, 1], f32, tag="sm4")
            nc.vector.reduce_sum(sm4, ex4, axis=AX.X)
            rc4 = rt.tile([P, NT4, 1], f32, tag="rc4")
            nc.vector.reciprocal(rc4, sm4)
            ge1 = rt.tile([P, NT4, E], f32, tag="ge1")
            nc.vector.tensor_scalar(ge1, lgs, 0.0, 1e30, OP.is_ge, OP.mult)
            lm4 = rt.tile([P, NT4, E], f32, tag="lm4")
            nc.vector.tensor_sub(lm4, lgs, ge1)
            m24 = rt.tile([P, NT4, 1], f32, tag="m24")
            nc.vector.reduce_max(m24, lm4, axis=AX.X)
            msk4 = rt.tile([P, NT4, E], f32, tag="msk4")
            nc.vector.tensor_tensor(msk4, lgs, m24.to_broadcast((P, NT4, E)),
                                    OP.is_ge)
            pw4 = rt.tile([P, NT4, E], f32, tag="pw4")
            nc.vector.tensor_mul(pw4, ex4, msk4)
            nc.vector.tensor_tensor(pw4, pw4, rc4.to_broadcast((P, NT4, E)),
                                    OP.mult)
            nc.sync.dma_start(
                ct[:, :].rearrange("(t4 p) e -> p t4 e", p=P), pw4)

            # ---- shared expert FFN (layer 2) ----
            for dm in range(DKO):
                ps2 = p2p.tile([P, TS], f32, tag="ps2")
                _dr_steps(nc, PM, ps2, sw2_sb, dm * P, (dm + 1) * P,
                          h8, 0, TS, HKO)
                o_sb = op_.tile([P, TS], f32, tag="o_sb")
                nc.scalar.activation(o_sb, ps2, AF.Identity,
                                     bias=sb2c_sb[:, dm:dm + 1],
                                     scale=scal_sb[:, 1:2])
                nc.sync.dma_start(sh[:, dm, :], o_sb)

    nc.finalize()
    return nc


def _build_phase2(C):
    import concourse.mybir as mybir
    import concourse.tile as tile
    from concourse import bacc

    f32 = mybir.dt.float32
    fp8 = mybir.dt.float8e4
    AF = mybir.ActivationFunctionType
    OP = mybir.AluOpType
    PM = mybir.MatmulPerfMode

    nc = bacc.Bacc("TRN2", target_bir_lowering=False, debug=False,
                   num_devices=NCORES)

    xg8 = nc.dram_tensor("xg8", [P, 2, DKO, C], fp8, kind="ExternalInput")
    w1q = nc.dram_tensor("w1q", [P, 2, DKO, H], fp8, kind="ExternalInput")
    b1c = nc.dram_tensor("b1c", [P, HKO], f32, kind="ExternalInput")
    w2q = nc.dram_tensor("w2q", [P, 2, HKO, DIM], fp8, kind="ExternalInput")
    b2c = nc.dram_tensor("b2c", [P, DKO], f32, kind="ExternalInput")
    ceg = nc.dram_tensor("ceg", [P, C], f32, kind="ExternalInput")
    scal = nc.dram_tensor("scal", [P, 2], f32, kind="ExternalInput")
    eo = nc.dram_tensor("eo", [P, DKO, C], f32, kind="ExternalOutput")

    SLICES2 = _slices_for(C)
    with tile.TileContext(nc) as tc:
        with (
            tc.tile_pool(name="const", bufs=1) as const,
            tc.tile_pool(name="wpool", bufs=1) as wpool,
            tc.tile_pool(name="gp", bufs=3) as gp,
            tc.tile_pool(name="hp", bufs=1) as hp,
            tc.tile_pool(name="op", bufs=1) as op_,
            tc.tile_pool(name="p1", bufs=4, space="PSUM") as p1p,
            tc.tile_pool(name="p2", bufs=3, space="PSUM") as p2p,
        ):
            # Startup: small first chunks so the first matmuls launch early
            xg_sb = wpool.tile([P, 2, DKO, C], fp8)
            cw0 = min(512, C)
            nc.sync.dma_start(xg_sb[:, :, :, 0:cw0], xg8[:, :, :, 0:cw0])
            w1_sb = wpool.tile([P, 2, DKO, H], fp8)
            nc.sync.dma_start(w1_sb[:, :, :, 0:128], w1q[:, :, :, 0:128])
            b1c_sb = const.tile([P, HKO], f32)
            nc.sync.dma_start(b1c_sb, b1c[:, :])
            scal_sb = const.tile([P, 2], f32)
            nc.sync.dma_start(scal_sb, scal[:, :])
            for h0 in range(128, H, 512):
                h1 = min(h0 + 512, H)
                nc.sync.dma_start(w1_sb[:, :, :, h0:h1],
                                  w1q[:, :, :, h0:h1])
            if C > cw0:
                nc.sync.dma_start(xg_sb[:, :, :, cw0:C], xg8[:, :, :, cw0:C])
            b2c_sb = const.tile([P, DKO], f32)
            nc.sync.dma_start(b2c_sb, b2c[:, :])
            ceg_sb = const.tile([P, C], f32)
            nc.sync.dma_start(ceg_sb, ceg[:, :])
            w2_sb = wpool.tile([P, 2, HKO, DIM], fp8)
            for d0 in range(0, DIM, 512):
                nc.sync.dma_start(w2_sb[:, :, :, d0:d0 + 512],
                                  w2q[:, :, :, d0:d0 + 512])

            c0 = 0
            for s, W in enumerate(SLICES2):
                h8 = hp.tile([P, 2, HKO, 512], fp8, tag="h8")
                for hm in range(HKO):
                    ps = p1p.tile([P, 512], f32, tag="ps1")
                    _dr_steps(nc, PM, ps[:, :W], w1_sb, hm * P,
                              (hm + 1) * P, xg_sb, c0, c0 + W, DKO)
                    g = gp.tile([P, 512], f32, tag="g")
                    nc.scalar.activation(g[:, :W], ps[:, :W], AF.Gelu,
                                         bias=b1c_sb[:, hm:hm + 1],
                                         scale=scal_sb[:, 0:1])
                    nc.gpsimd.tensor_copy(h8[:, 0, hm, :W], g[:, :W])
                    nc.vector.scalar_tensor_tensor(
                        h8[:, 1, hm, :W], g[:, :W], 1.0, h8[:, 0, hm, :W],
                        OP.mult, OP.subtract)
                ob = op_.tile([P, DKO, 512], f32, tag="ob")
                for dm in range(DKO):
                    ps2 = p2p.tile([P, 512], f32, tag="ps2")
                    _dr_steps(nc, PM, ps2[:, :W], w2_sb, dm * P,
                              (dm + 1) * P, h8, 0, W, HKO)
                    nc.scalar.activation(ob[:, dm, :W], ps2[:, :W],
                                         AF.Identity,
                                         bias=b2c_sb[:, dm:dm + 1],
                                         scale=scal_sb[:, 1:2])
                    nc.vector.tensor_mul(ob[:, dm, :W], ob[:, dm, :W],
                                         ceg_sb[:, c0:c0 + W])
                nc.sync.dma_start(eo[:, :, c0:c0 + W], ob[:, :, :W])
                c0 += W

    nc.finalize()
    return nc


def _get(name, builder):
    if name not in _nc_cache:
        _nc_cache[name] = builder()
    return _nc_cache[name]


def _run_spmd(nc, in_maps):
    from concourse.bass_utils import run_bass_kernel_spmd
    return run_bass_kernel_spmd(nc, in_maps, core_ids=list(range(NCORES)))


def _gelu_np(v):
    from scipy.special import erf
    return 0.5 * v * (1.0 + erf(v / np.sqrt(2.0)))


def kernel(x, router_w, router_b, w1, b1, w2, b2, sw1, sb1, sw2, sb2):
    x = np.asarray(x, F32)
    xt = np.ascontiguousarray(x.reshape(T, DIM).T)          # [DIM, T]
    xt32p = np.ascontiguousarray(
        xt.reshape(DKO, P, T).transpose(1, 0, 2))           # [P, DKO, T]
    xhi, xlo = _split_fp8(xt)
    x8p = np.empty((P, 2, DKO, T), E4M3)
    x8p[:, 0] = xhi.reshape(DKO, P, T).transpose(1, 0, 2)
    x8p[:, 1] = xlo.reshape(DKO, P, T).transpose(1, 0, 2)

    rwp = np.ascontiguousarray(
        np.asarray(router_w, F32).reshape(DKO, P, E).transpose(1, 0, 2))
    rbp = np.ascontiguousarray(
        np.tile(np.asarray(router_b, F32)[None, :], (P, 1)))

    sw1q, s1 = _quant_w(np.asarray(sw1, F32))
    sw2q, s2 = _quant_w(np.asarray(sw2, F32))
    sb1cp = np.ascontiguousarray(np.asarray(sb1, F32).reshape(HKO, P).T)
    sb2cp = np.ascontiguousarray(np.asarray(sb2, F32).reshape(DKO, P).T)
    sscal = np.ascontiguousarray(np.tile(np.array([[s1, s2]], F32), (P, 1)))

    maps1 = []
    for j in range(NCORES):
        t0 = j * TS
        maps1.append(dict(
            xs32=np.ascontiguousarray(xt32p[:, :, t0:t0 + TS]),
            xs8=np.ascontiguousarray(x8p[:, :, :, t0:t0 + TS]),
            rwp=rwp, rb=rbp, sw1q=sw1q, sb1c=sb1cp, sw2q=sw2q,
            sb2c=sb2cp, scal=sscal))
    res1 = _run_spmd(_get("p1", _build_phase1), maps1)

    ctf = np.concatenate([r["ct"] for r in res1.results], axis=0)  # [T, E]
    shared = np.empty((T, DIM), F32)
    for j, r in enumerate(res1.results):
        # sh [P, DKO, TS]: value (ki, dm, t) = shared[dim dm*128+ki, tok]
        shj = r["sh"].transpose(2, 1, 0).reshape(TS, DIM)
        shared[j * TS:(j + 1) * TS] = shj

    # ---- host all-to-all dispatch by top-k expert id ----
    counts = [int((ctf[:, e] != 0).sum()) for e in range(NCORES)]
    C = max(32, ((max(counts) + CAP_QUANT - 1) // CAP_QUANT) * CAP_QUANT)
    maps2 = []
    sels = []
    for e in range(NCORES):
        sel = np.nonzero(ctf[:, e])[0]
        sels.append(sel)
        npad = C - len(sel)
        selp = np.concatenate([sel, np.zeros(npad, np.int64)])
        cev = np.concatenate([ctf[sel, e].astype(F32), np.zeros(npad, F32)])
        w1qe, e1 = _quant_w(np.asarray(w1[e], F32))
        w2qe, e2 = _quant_w(np.asarray(w2[e], F32))
        maps2.append(dict(
            xg8=np.ascontiguousarray(x8p[:, :, :, selp]),
            w1q=w1qe,
            b1c=np.ascontiguousarray(np.asarray(b1[e], F32).reshape(HKO, P).T),
            w2q=w2qe,
            b2c=np.ascontiguousarray(np.asarray(b2[e], F32).reshape(DKO, P).T),
            ceg=np.ascontiguousarray(np.tile(cev[None, :], (P, 1))),
            scal=np.ascontiguousarray(np.tile(np.array([[e1, e2]], F32), (P, 1)))))
    res2 = _run_spmd(_get(("p2", C), lambda: _build_phase2(C)), maps2)
    _nc_cache["p2_last"] = _nc_cache[("p2", C)]

    out = shared
    for e, r in enumerate(res2.results):
        n = len(sels[e])
        # eo [P, DKO, C] -> token-major [C, DIM]
        ye = r["eo"].transpose(2, 1, 0).reshape(C, DIM)
        out[sels[e]] += ye[:n]

    return out.reshape(2, 2048, DIM)
